# revision 14
# baseline (speedup 1.0000x reference)
"""Trainium2 Bass kernel for nn_BoundleAdjustment (2M observations).

Two launches on all 8 NeuronCores (observations data-parallel, M/8 per core):

Launch A (device): converts the 4096-row pose table (translation+quaternion)
into per-pose rotation matrices R = f(q/|q|) on the Vector engine
([128, 32] planar layout, one reciprocal for the 2/|q|^2 scale).

Host staging (indexing/layout only): gathers the derived R table, raw pose
translations, and patch rows by poses_idx/patch_idx, casts the per-
observation record planes to fp16, and lays them out as two contiguous
blocks per chunk so each chunk needs only two big DMAs.

Launch B (device): streams fp16 planes through SBUF in 2 chunks.
Rotation + residual math runs in fp16 on the Vector engine (2x DVE mode);
squares/sqrts/arctans on the Scalar engine; the azimuth uses the
half-angle identity az = 2*atan(ry/(rho+rx)) which needs no quadrant
fixup; the two reciprocals run in f32 via reciprocal_approx_fast with
max(x,1e-30) guards so no inf/NaN can form.
"""

import numpy as np

M = 2097152
NCORES = 8
N = M // NCORES
P = 128
COLS = N // P            # 2048
CC = 1024                # chunk cols
NCH = COLS // CC         # 2 chunks
NPOSE = 4096
PC = NPOSE // P          # 32 cols for pose table

# plane groups: A holds the rot-x chain (computed first), B the rest.
# Pose translations arrive via gpsimd accumulate-DMAs onto the row sums,
# negated targets via accumulate-DMAs onto rng/az2/el.
NPA = 6                  # R00 R01 R02 px py pz
NPB_ = 7                 # R10 R11 R12 R20 R21 R22 W

_CACHE = {}


# launch A staged layout: 22 blocks of 32 cols, products prod_k = QA_k * QB_k
#   0-5   PL1 = yy xx xx xy xz yz      6-11  PL2 = zz zz yy wz wy wx
#   12-14 MN1 = xy xz yz               15-17 MN2 = wz wy wx
#   18-21 SS  = xx yy zz ww
# plus  = PL1+PL2 = [d00 d11 d22 o10 o02 o21], minus = MN1-MN2 = [o01 o20 o12]
_QA_IDX = [1, 0, 0, 0, 0, 1,  2, 2, 1, 3, 3, 3,  0, 0, 1,  3, 3, 3,  0, 1, 2, 3]
_QB_IDX = [1, 0, 0, 1, 2, 2,  2, 2, 1, 2, 1, 0,  1, 2, 2,  2, 1, 0,  0, 1, 2, 3]
NQB = 22


def _build_posetab():
    import concourse.tile as tile
    from concourse import bacc, mybir

    nc = bacc.Bacc("TRN2", target_bir_lowering=False, debug=False,
                   num_devices=NCORES)
    f32 = mybir.dt.float32
    OP = mybir.AluOpType
    qa_d = nc.declare_dram_parameter("qa", [P, NQB * PC], f32, isOutput=False)
    qb_d = nc.declare_dram_parameter("qb", [P, NQB * PC], f32, isOutput=False)
    r_d = nc.declare_dram_parameter("rtab", [P, 9 * PC], f32, isOutput=True)

    with tile.TileContext(nc) as tc:
        with tc.tile_pool(name="pp", bufs=12) as pp:
            vec = nc.vector
            qa = pp.tile([P, NQB * PC], f32, tag="qa", name="qa")
            nc.sync.dma_start(qa[:], qa_d[:, :])
            qb = pp.tile([P, NQB * PC], f32, tag="qb", name="qb")
            nc.sync.dma_start(qb[:], qb_d[:, :])
            rt = pp.tile([P, 9 * PC], f32, tag="rt", name="rt")

            def blk(t, i, n=1):
                return t[:, i * PC:(i + n) * PC]

            prod = pp.tile([P, NQB * PC], f32, tag="prod", name="prod")
            vec.tensor_tensor(out=prod[:], in0=qa[:], in1=qb[:], op=OP.mult)
            plus = pp.tile([P, 6 * PC], f32, tag="plus", name="plus")
            vec.tensor_tensor(out=plus[:], in0=blk(prod, 0, 6),
                              in1=blk(prod, 6, 6), op=OP.add)
            minus = pp.tile([P, 3 * PC], f32, tag="minus", name="minus")
            vec.tensor_tensor(out=minus[:], in0=blk(prod, 12, 3),
                              in1=blk(prod, 15, 3), op=OP.subtract)
            s2 = pp.tile([P, 2 * PC], f32, tag="s2", name="s2")
            vec.tensor_tensor(out=s2[:], in0=blk(prod, 18, 2),
                              in1=blk(prod, 20, 2), op=OP.add)
            d1 = pp.tile([P, PC], f32, tag="d1", name="d1")
            # d1 = 0.5*(xx+yy) + 0.5*(zz+ww) via STT: (a*0.5) + b*... do in 2
            vec.tensor_tensor(out=d1[:], in0=blk(s2, 0), in1=blk(s2, 1),
                              op=OP.add)
            dh = pp.tile([P, PC], f32, tag="dh", name="dh")
            vec.tensor_scalar(out=dh[:], in0=d1[:], scalar1=0.5, scalar2=None,
                              op0=OP.mult)
            u = pp.tile([P, PC], f32, tag="u", name="u")
            vec.reciprocal(u[:], dh[:])        # u = 2/|q|^2

            # off-diagonals: R order R00 R01 R02 R10 R11 R12 R20 R21 R22
            for src, dst in ((3, 3), (4, 2), (5, 7)):      # plus -> o10 o02 o21
                vec.tensor_tensor(out=blk(rt, dst), in0=blk(plus, src),
                                  in1=u[:], op=OP.mult)
            for src, dst in ((0, 1), (1, 6), (2, 5)):      # minus -> o01 o20 o12
                vec.tensor_tensor(out=blk(rt, dst), in0=blk(minus, src),
                                  in1=u[:], op=OP.mult)
            # diagonals: R_ii = 1 - u*(pair)
            dgm = pp.tile([P, 3 * PC], f32, tag="dgm", name="dgm")
            for i in range(3):
                vec.tensor_tensor(out=blk(dgm, i), in0=blk(plus, i),
                                  in1=u[:], op=OP.mult)
            for i, dst in enumerate((0, 4, 8)):
                vec.tensor_scalar(out=blk(rt, dst), in0=blk(dgm, i),
                                  scalar1=-1.0, scalar2=1.0,
                                  op0=OP.mult, op1=OP.add)
            nc.sync.dma_start(r_d[:, :], rt[:])
    nc.finalize()
    return nc


def _build_main():
    import concourse.tile as tile
    from concourse import bacc, mybir

    nc = bacc.Bacc("TRN2", target_bir_lowering=False, debug=False,
                   num_devices=NCORES)
    f16 = mybir.dt.float16
    f32 = mybir.dt.float32
    AF = mybir.ActivationFunctionType
    OP = mybir.AluOpType
    inA_d = nc.declare_dram_parameter("inA", [NCH, P, NPA * CC], f16,
                                      isOutput=False)
    inB_d = nc.declare_dram_parameter("inB", [NCH, P, NPB_ * CC], f16,
                                      isOutput=False)
    t_d = nc.declare_dram_parameter("tadd", [NCH, 3, P, CC], f16,
                                    isOutput=False)
    g_d = nc.declare_dram_parameter("ntgt", [NCH, 3, P, CC], f16,
                                    isOutput=False)
    out_d = nc.declare_dram_parameter("out", [NCH, P, 3 * CC], f16,
                                      isOutput=True)

    with tile.TileContext(nc) as tc:
        with tc.tile_pool(name="inp", bufs=2) as inp, \
             tc.tile_pool(name="outp", bufs=2) as outp, \
             tc.tile_pool(name="t16p", bufs=30) as t16p, \
             tc.tile_pool(name="t32p", bufs=12) as t32p:
            vec, act = nc.vector, nc.scalar
            for ch in range(NCH):
                tA = inp.tile([P, NPA * CC], f16, tag="inA", name=f"inA{ch}")
                nc.sync.dma_start(tA[:], inA_d[ch])
                tB = inp.tile([P, NPB_ * CC], f16, tag="inB", name=f"inB{ch}")
                nc.sync.dma_start(tB[:], inB_d[ch])
                ot = outp.tile([P, 3 * CC], f16, tag="out", name=f"out{ch}")

                vA = lambda k: tA[:, k * CC:(k + 1) * CC]
                vB = lambda k: tB[:, k * CC:(k + 1) * CC]
                R00, R01, R02, PX, PY, PZ = (vA(k) for k in range(6))
                R10, R11, R12 = (vB(k) for k in range(3))
                R20, R21, R22 = (vB(k) for k in range(3, 6))
                W = vB(6)

                cnt = [0]

                def T16():
                    cnt[0] += 1
                    return t16p.tile([P, CC], f16, tag="t16",
                                     name=f"t16_{ch}_{cnt[0]}")

                def T32():
                    cnt[0] += 1
                    return t32p.tile([P, CC], f32, tag="t32",
                                     name=f"t32_{ch}_{cnt[0]}")

                def tt(a, b, op):
                    d = T16()
                    vec.tensor_tensor(out=d[:], in0=a, in1=b, op=op)
                    return d

                def ap(x):
                    return x[:] if hasattr(x, "tile_id") else x

                def row(Ra, Rb, Rc, k):
                    m0 = tt(Ra, PX, OP.mult)
                    m1 = tt(Rb, PY, OP.mult)
                    m2 = tt(Rc, PZ, OP.mult)
                    a0 = tt(m0[:], m1[:], OP.add)
                    r = tt(a0[:], m2[:], OP.add)
                    # r += t (pose translation) during delivery (CCE add)
                    nc.gpsimd.dma_start(r[:], t_d[ch, k], accum_op=OP.add)
                    return r

                rx = row(R00, R01, R02, 0)
                ry = row(R10, R11, R12, 1)
                rz = row(R20, R21, R22, 2)

                def sq(a):
                    d = T16()
                    act.activation(d[:], a[:], AF.Square)
                    return d

                sqx, sqy, sqz = sq(rx), sq(ry), sq(rz)
                rho2 = tt(sqx[:], sqy[:], OP.add)
                r2 = tt(rho2[:], sqz[:], OP.add)
                rng = T16(); act.activation(rng[:], r2[:], AF.Sqrt)
                rho = T32(); act.activation(rho[:], rho2[:], AF.Sqrt)

                rho_g = T32()
                vec.tensor_scalar(out=rho_g[:], in0=rho[:], scalar1=1e-30,
                                  scalar2=None, op0=OP.max)
                irho = T32(); vec.reciprocal_approx_fast(irho[:], rho_g[:])
                den = T32()
                vec.tensor_tensor(out=den[:], in0=rho_g[:], in1=rx[:],
                                  op=OP.add)
                den_g = T32()
                vec.tensor_scalar(out=den_g[:], in0=den[:], scalar1=1e-30,
                                  scalar2=None, op0=OP.max)
                iden = T32(); vec.reciprocal_approx_fast(iden[:], den_g[:])
                q = T32()
                vec.tensor_tensor(out=q[:], in0=ry[:], in1=iden[:], op=OP.mult)
                e1 = T32()
                vec.tensor_tensor(out=e1[:], in0=rz[:], in1=irho[:],
                                  op=OP.mult)
                az0 = T16(); act.activation(az0[:], q[:], AF.Arctan)
                el = T16(); act.activation(el[:], e1[:], AF.Arctan)
                az2 = T16()
                vec.tensor_scalar(out=az2[:], in0=az0[:], scalar1=2.0,
                                  scalar2=None, op0=OP.mult)

                # proj -= target via CCE add of staged -target planes,
                # then weight multiply on gpsimd into the output tile
                for k, t in enumerate((rng, az2, el)):
                    nc.gpsimd.dma_start(t[:], g_d[ch, k], accum_op=OP.add)
                    nc.gpsimd.tensor_tensor(out=ot[:, k * CC:(k + 1) * CC],
                                            in0=t[:], in1=W, op=OP.mult)
                nc.sync.dma_start(out_d[ch], ot[:])
    nc.finalize()
    return nc


def _get(name, builder):
    if name not in _CACHE:
        _CACHE[name] = builder()
    return _CACHE[name]


def stage_q(poses):
    """(qa, qb) [128, NQB*32] f32 operand planes for launch A's one big mult."""
    qp = poses[:, 3:7].reshape(P, PC, 4).transpose(2, 0, 1)  # [4,128,32]
    qa = np.concatenate([qp[i] for i in _QA_IDX], axis=1)
    qb = np.concatenate([qp[i] for i in _QB_IDX], axis=1)
    return np.ascontiguousarray(qa), np.ascontiguousarray(qb)


def decode_rtab(raw):
    """[128, 9*32] device layout -> [4096, 9] table."""
    r = np.asarray(raw).reshape(P, 9, PC).transpose(0, 2, 1)  # [128, 32, 9]
    return np.ascontiguousarray(r.reshape(NPOSE, 9))


def stage_obs(rtab, poses, patch_coords, elevation_angle, pid, qid,
              target_coords, weights):
    """Gather per-observation planes, cast fp16, lay out per core/chunk.

    Returns (bigA, bigB, tadd [NCORES,NCH,3,P,CC], ntgt [same]) all f16.
    """
    r9 = rtab[pid]                                            # [M, 9]
    t3 = poses[pid, 0:3]                                      # [M, 3]
    pts = np.concatenate(
        [patch_coords[qid], elevation_angle[qid]], axis=1)    # [M, 3]
    valA = np.concatenate([r9[:, 0:3], pts], axis=1)
    valB = np.concatenate([r9[:, 3:9], weights], axis=1)

    def lay(v, np_):
        v = v.astype(np.float16)
        v = v.reshape(NCORES, P, NCH, CC, np_).transpose(0, 2, 1, 4, 3)
        return np.ascontiguousarray(v)

    def lay_t(v):
        v = v.astype(np.float16)
        v = v.reshape(NCORES, P, NCH, CC, 3).transpose(0, 2, 4, 1, 3)
        return np.ascontiguousarray(v)

    return (lay(valA, NPA), lay(valB, NPB_), lay_t(t3),
            lay_t(-target_coords))


def unstage_out(res_list):
    """res_list: per-core [NCH,P,3,CC] f16 -> [M,3] f32."""
    out = np.stack([np.asarray(r).reshape(NCH, P, 3, CC) for r in res_list])
    out = out.transpose(0, 2, 1, 4, 3).reshape(M, 3)
    return np.ascontiguousarray(out).astype(np.float32)


def kernel(poses, patch_coords, elevation_angle, poses_idx, patch_idx,
           target_coords, weights):
    from concourse.bass_utils import run_bass_kernel_spmd

    poses = np.asarray(poses, dtype=np.float32)
    patch_coords = np.asarray(patch_coords, dtype=np.float32)
    elevation_angle = np.asarray(elevation_angle, dtype=np.float32)
    target_coords = np.asarray(target_coords, dtype=np.float32)
    weights = np.asarray(weights, dtype=np.float32)
    pid = np.asarray(poses_idx).astype(np.int64)
    qid = np.asarray(patch_idx).astype(np.int64)

    # ---- launch A: pose table -> rotation matrices (device) ----
    qa, qb = stage_q(poses)
    ncA = _get("A", _build_posetab)
    resA = run_bass_kernel_spmd(ncA,
                                [{"qa": qa, "qb": qb} for _ in range(NCORES)],
                                list(range(NCORES)))
    rtab = decode_rtab(resA.results[0]["rtab"])

    # ---- host: gather + fp16 staging (indexing/layout only) ----
    bigA, bigB, tadd, ntgt = stage_obs(rtab, poses, patch_coords,
                                       elevation_angle, pid, qid,
                                       target_coords, weights)

    # ---- launch B: streaming rotate+polar+residual ----
    ncB = _get("B", _build_main)
    resB = run_bass_kernel_spmd(
        ncB, [{"inA": bigA[c], "inB": bigB[c], "tadd": tadd[c],
               "ntgt": ntgt[c]} for c in range(NCORES)],
        list(range(NCORES)))
    return unstage_out([resB.results[c]["out"] for c in range(NCORES)])


# revision 18
# speedup vs baseline: 1.1393x; 1.1393x over previous
"""Trainium2 Bass kernel for nn_BoundleAdjustment (2M observations).

Two launches on all 8 NeuronCores (observations data-parallel, M/8 per core):

Launch A (device): converts the 4096-row pose table (translation+quaternion)
into per-pose rotation matrices R = f(q/|q|) on the Vector engine
([128, 32] planar layout, one reciprocal for the 2/|q|^2 scale).

Host staging (indexing/layout only): gathers the derived R table, raw pose
translations, and patch rows by poses_idx/patch_idx, casts the per-
observation record planes to fp16, and lays them out as two contiguous
blocks per chunk so each chunk needs only two big DMAs.

Launch B (device): streams fp16 planes through SBUF in 2 chunks.
Rotation + residual math runs in fp16 on the Vector engine (2x DVE mode);
squares/sqrts/arctans on the Scalar engine; the azimuth uses the
half-angle identity az = 2*atan(ry/(rho+rx)) which needs no quadrant
fixup; the two reciprocals run in f32 via reciprocal_approx_fast with
max(x,1e-30) guards so no inf/NaN can form.
"""

import numpy as np

M = 2097152
NCORES = 8
N = M // NCORES
P = 128
COLS = N // P            # 2048
CC = 1024                # chunk cols
NCH = COLS // CC         # 2 chunks
NPOSE = 4096
PC = NPOSE // P          # 32 cols for pose table

# plane groups (fp16):
# tA: px py pz | tx ty tz | W          (7 planes)
# tB: R00 R01 R02 R10 R11 R12 R20 R21 R22 | X Y Z   (12 planes)
NPA = 7
NPB_ = 12

_CACHE = {}


# launch A staged layout: 22 blocks of 32 cols, products prod_k = QA_k * QB_k
#   0-5   PL1 = yy xx xx xy xz yz      6-11  PL2 = zz zz yy wz wy wx
#   12-14 MN1 = xy xz yz               15-17 MN2 = wz wy wx
#   18-21 SS  = xx yy zz ww
# plus  = PL1+PL2 = [d00 d11 d22 o10 o02 o21], minus = MN1-MN2 = [o01 o20 o12]
_QA_IDX = [1, 0, 0, 0, 0, 1,  2, 2, 1, 3, 3, 3,  0, 0, 1,  3, 3, 3,  0, 1, 2, 3]
_QB_IDX = [1, 0, 0, 1, 2, 2,  2, 2, 1, 2, 1, 0,  1, 2, 2,  2, 1, 0,  0, 1, 2, 3]
NQB = 22


def _build_posetab():
    import concourse.tile as tile
    from concourse import bacc, mybir

    nc = bacc.Bacc("TRN2", target_bir_lowering=False, debug=False,
                   num_devices=NCORES)
    f32 = mybir.dt.float32
    OP = mybir.AluOpType
    qa_d = nc.declare_dram_parameter("qa", [P, NQB * PC], f32, isOutput=False)
    qb_d = nc.declare_dram_parameter("qb", [P, NQB * PC], f32, isOutput=False)
    r_d = nc.declare_dram_parameter("rtab", [P, 9 * PC], f32, isOutput=True)

    with tile.TileContext(nc) as tc:
        with tc.tile_pool(name="pp", bufs=12) as pp:
            vec = nc.vector
            qa = pp.tile([P, NQB * PC], f32, tag="qa", name="qa")
            nc.sync.dma_start(qa[:], qa_d[:, :])
            qb = pp.tile([P, NQB * PC], f32, tag="qb", name="qb")
            nc.sync.dma_start(qb[:], qb_d[:, :])
            rt = pp.tile([P, 9 * PC], f32, tag="rt", name="rt")

            def blk(t, i, n=1):
                return t[:, i * PC:(i + n) * PC]

            prod = pp.tile([P, NQB * PC], f32, tag="prod", name="prod")
            vec.tensor_tensor(out=prod[:], in0=qa[:], in1=qb[:], op=OP.mult)
            plus = pp.tile([P, 6 * PC], f32, tag="plus", name="plus")
            vec.tensor_tensor(out=plus[:], in0=blk(prod, 0, 6),
                              in1=blk(prod, 6, 6), op=OP.add)
            minus = pp.tile([P, 3 * PC], f32, tag="minus", name="minus")
            vec.tensor_tensor(out=minus[:], in0=blk(prod, 12, 3),
                              in1=blk(prod, 15, 3), op=OP.subtract)
            s2 = pp.tile([P, 2 * PC], f32, tag="s2", name="s2")
            vec.tensor_tensor(out=s2[:], in0=blk(prod, 18, 2),
                              in1=blk(prod, 20, 2), op=OP.add)
            d1 = pp.tile([P, PC], f32, tag="d1", name="d1")
            # d1 = 0.5*(xx+yy) + 0.5*(zz+ww) via STT: (a*0.5) + b*... do in 2
            vec.tensor_tensor(out=d1[:], in0=blk(s2, 0), in1=blk(s2, 1),
                              op=OP.add)
            dh = pp.tile([P, PC], f32, tag="dh", name="dh")
            vec.tensor_scalar(out=dh[:], in0=d1[:], scalar1=0.5, scalar2=None,
                              op0=OP.mult)
            u = pp.tile([P, PC], f32, tag="u", name="u")
            vec.reciprocal(u[:], dh[:])        # u = 2/|q|^2

            # off-diagonals: R order R00 R01 R02 R10 R11 R12 R20 R21 R22
            for src, dst in ((3, 3), (4, 2), (5, 7)):      # plus -> o10 o02 o21
                vec.tensor_tensor(out=blk(rt, dst), in0=blk(plus, src),
                                  in1=u[:], op=OP.mult)
            for src, dst in ((0, 1), (1, 6), (2, 5)):      # minus -> o01 o20 o12
                vec.tensor_tensor(out=blk(rt, dst), in0=blk(minus, src),
                                  in1=u[:], op=OP.mult)
            # diagonals: R_ii = 1 - u*(pair)
            dgm = pp.tile([P, 3 * PC], f32, tag="dgm", name="dgm")
            for i in range(3):
                vec.tensor_tensor(out=blk(dgm, i), in0=blk(plus, i),
                                  in1=u[:], op=OP.mult)
            for i, dst in enumerate((0, 4, 8)):
                vec.tensor_scalar(out=blk(rt, dst), in0=blk(dgm, i),
                                  scalar1=-1.0, scalar2=1.0,
                                  op0=OP.mult, op1=OP.add)
            nc.sync.dma_start(r_d[:, :], rt[:])
    nc.finalize()
    return nc


def _build_main():
    import concourse.tile as tile
    from concourse import bacc, mybir

    nc = bacc.Bacc("TRN2", target_bir_lowering=False, debug=False,
                   num_devices=NCORES)
    f16 = mybir.dt.float16
    f32 = mybir.dt.float32
    AF = mybir.ActivationFunctionType
    OP = mybir.AluOpType
    inA_d = nc.declare_dram_parameter("inA", [NCH, P, NPA * CC], f16,
                                      isOutput=False)
    inB_d = nc.declare_dram_parameter("inB", [NCH, P, NPB_ * CC], f16,
                                      isOutput=False)
    out_d = nc.declare_dram_parameter("out", [NCH, P, 3 * CC], f16,
                                      isOutput=True)

    with tile.TileContext(nc) as tc:
        with tc.tile_pool(name="inp", bufs=2) as inp, \
             tc.tile_pool(name="tp", bufs=2) as tp:
            vec, act = nc.vector, nc.scalar
            for ch in range(NCH):
                tA = inp.tile([P, NPA, CC], f16, tag="inA", name=f"inA{ch}")
                nc.sync.dma_start(tA[:], inA_d[ch])
                tB = inp.tile([P, NPB_, CC], f16, tag="inB", name=f"inB{ch}")
                nc.sync.dma_start(tB[:], inB_d[ch])
                ot = tp.tile([P, 3 * CC], f16, tag="out", name=f"out{ch}")

                def TL(shape, dt, tag, bufs):
                    return tp.tile(shape, dt, tag=tag, name=f"{tag}{ch}",
                                   bufs=bufs)

                P3 = tA[:, 0:3, :]                      # px py pz
                T3 = tA[:, 3:6, :]                      # tx ty tz
                Wp = tA[:, 6, :]                        # weights
                R9 = tB[:, 0:9, :]
                X, Y, Z = tB[:, 9, :], tB[:, 10, :], tB[:, 11, :]

                # all 9 rotation products in one op: R9 * [p3 p3 p3]
                m9 = TL([P, 9, CC], f16, "m9", 1)
                p3b = tA[:, 0:3, :].unsqueeze(1).broadcast_to([P, 3, 3, CC])
                vec.tensor_tensor(out=m9[:], in0=R9, in1=p3b, op=OP.mult)
                # row sums + translation: r3 = [rx ry rz]
                s1 = TL([P, 3, CC], f16, "s1", 1)
                vec.tensor_tensor(out=s1[:], in0=m9[:, 0::3, :],
                                  in1=m9[:, 1::3, :], op=OP.add)
                s2 = TL([P, 3, CC], f16, "s2", 1)
                vec.tensor_tensor(out=s2[:], in0=s1[:], in1=m9[:, 2::3, :],
                                  op=OP.add)
                r3 = TL([P, 3, CC], f16, "r3", 2)
                vec.tensor_tensor(out=r3[:], in0=s2[:], in1=T3, op=OP.add)
                rx, ry, rz = r3[:, 0, :], r3[:, 1, :], r3[:, 2, :]

                # squares (one op), rho2/r2 into one packed pair, sqrt pair
                sq3 = TL([P, 3, CC], f16, "sq3", 2)
                act.activation(sq3[:], r3[:], AF.Square)
                rr = TL([P, 2, CC], f16, "rr", 2)
                vec.tensor_tensor(out=rr[:, 0, :], in0=sq3[:, 0, :],
                                  in1=sq3[:, 1, :], op=OP.add)
                vec.tensor_tensor(out=rr[:, 1, :], in0=rr[:, 0, :],
                                  in1=sq3[:, 2, :], op=OP.add)
                sr = TL([P, 2, CC], f16, "sr", 2)
                act.activation(sr[:], rr[:], AF.Sqrt)   # [rho | rng]
                rho, rng = sr[:, 0, :], sr[:, 1, :]

                # guarded reciprocals in f32 (no inf/NaN possible)
                rho_g = TL([P, CC], f32, "rhog", 1)
                vec.tensor_scalar(out=rho_g[:], in0=rho, scalar1=1e-30,
                                  scalar2=None, op0=OP.max)
                irho = TL([P, CC], f32, "irho", 1)
                vec.reciprocal_approx_fast(irho[:], rho_g[:])
                den = TL([P, CC], f32, "den", 1)
                vec.tensor_tensor(out=den[:], in0=rho_g[:], in1=rx, op=OP.add)
                den_g = TL([P, CC], f32, "deng", 1)
                vec.tensor_scalar(out=den_g[:], in0=den[:], scalar1=1e-30,
                                  scalar2=None, op0=OP.max)
                iden = TL([P, CC], f32, "iden", 1)
                vec.reciprocal_approx_fast(iden[:], den_g[:])
                # atan args packed: [ry/(rho+rx) | rz/rho]
                qa2 = TL([P, 2, CC], f32, "qa2", 1)
                vec.tensor_tensor(out=qa2[:, 0, :], in0=ry, in1=iden[:],
                                  op=OP.mult)
                vec.tensor_tensor(out=qa2[:, 1, :], in0=rz, in1=irho[:],
                                  op=OP.mult)
                at2 = TL([P, 2, CC], f16, "at2", 2)
                act.activation(at2[:], qa2[:], AF.Arctan)  # [az0 | el]

                # residuals: d3 = [rng-X | 2*az0-Y | el-Z], out = d3*W
                d3 = TL([P, 3, CC], f16, "d3", 1)
                vec.tensor_tensor(out=d3[:, 0, :], in0=rng, in1=X,
                                  op=OP.subtract)
                vec.scalar_tensor_tensor(out=d3[:, 1, :], in0=at2[:, 0, :],
                                         scalar=2.0, in1=Y,
                                         op0=OP.mult, op1=OP.subtract)
                vec.tensor_tensor(out=d3[:, 2, :], in0=at2[:, 1, :], in1=Z,
                                  op=OP.subtract)
                wb = Wp.unsqueeze(1).broadcast_to([P, 3, CC])
                vec.tensor_tensor(out=ot[:], in0=d3[:], in1=wb, op=OP.mult)
                nc.sync.dma_start(out_d[ch], ot[:])
    nc.finalize()
    return nc


def _get(name, builder):
    if name not in _CACHE:
        _CACHE[name] = builder()
    return _CACHE[name]


def stage_q(poses):
    """(qa, qb) [128, NQB*32] f32 operand planes for launch A's one big mult."""
    qp = poses[:, 3:7].reshape(P, PC, 4).transpose(2, 0, 1)  # [4,128,32]
    qa = np.concatenate([qp[i] for i in _QA_IDX], axis=1)
    qb = np.concatenate([qp[i] for i in _QB_IDX], axis=1)
    return np.ascontiguousarray(qa), np.ascontiguousarray(qb)


def decode_rtab(raw):
    """[128, 9*32] device layout -> [4096, 9] table."""
    r = np.asarray(raw).reshape(P, 9, PC).transpose(0, 2, 1)  # [128, 32, 9]
    return np.ascontiguousarray(r.reshape(NPOSE, 9))


def stage_obs(rtab, poses, patch_coords, elevation_angle, pid, qid,
              target_coords, weights):
    """Gather per-observation planes, cast fp16, lay out per core/chunk.

    Returns (bigA [NCORES,NCH,P,NPA,CC], bigB [NCORES,NCH,P,NPB_,CC]) f16.
    """
    r9 = rtab[pid]                                            # [M, 9]
    t3 = poses[pid, 0:3]                                      # [M, 3]
    pts = np.concatenate(
        [patch_coords[qid], elevation_angle[qid]], axis=1)    # [M, 3]
    valA = np.concatenate([pts, t3, weights], axis=1)
    valB = np.concatenate([r9, target_coords], axis=1)

    def lay(v, np_):
        v = v.astype(np.float16)
        v = v.reshape(NCORES, P, NCH, CC, np_).transpose(0, 2, 1, 4, 3)
        return np.ascontiguousarray(v)

    return lay(valA, NPA), lay(valB, NPB_)


def unstage_out(res_list):
    """res_list: per-core [NCH,P,3,CC] f16 -> [M,3] f32."""
    out = np.stack([np.asarray(r).reshape(NCH, P, 3, CC) for r in res_list])
    out = out.transpose(0, 2, 1, 4, 3).reshape(M, 3)
    return np.ascontiguousarray(out).astype(np.float32)


def kernel(poses, patch_coords, elevation_angle, poses_idx, patch_idx,
           target_coords, weights):
    from concourse.bass_utils import run_bass_kernel_spmd

    poses = np.asarray(poses, dtype=np.float32)
    patch_coords = np.asarray(patch_coords, dtype=np.float32)
    elevation_angle = np.asarray(elevation_angle, dtype=np.float32)
    target_coords = np.asarray(target_coords, dtype=np.float32)
    weights = np.asarray(weights, dtype=np.float32)
    pid = np.asarray(poses_idx).astype(np.int64)
    qid = np.asarray(patch_idx).astype(np.int64)

    # ---- launch A: pose table -> rotation matrices (device) ----
    qa, qb = stage_q(poses)
    ncA = _get("A", _build_posetab)
    resA = run_bass_kernel_spmd(ncA,
                                [{"qa": qa, "qb": qb} for _ in range(NCORES)],
                                list(range(NCORES)))
    rtab = decode_rtab(resA.results[0]["rtab"])

    # ---- host: gather + fp16 staging (indexing/layout only) ----
    bigA, bigB = stage_obs(rtab, poses, patch_coords, elevation_angle,
                           pid, qid, target_coords, weights)

    # ---- launch B: streaming rotate+polar+residual ----
    ncB = _get("B", _build_main)
    resB = run_bass_kernel_spmd(
        ncB, [{"inA": bigA[c], "inB": bigB[c]} for c in range(NCORES)],
        list(range(NCORES)))
    return unstage_out([resB.results[c]["out"] for c in range(NCORES)])
